# revision 3
# baseline (speedup 1.0000x reference)
"""Causal self-attention + output projection + residual + LayerNorm on 8
Trainium2 NeuronCores.

Problem: B=4, S=2048, D=1024, H=16, dk=64 (fp32).

Sharding: core c = 2*b + g handles batch b with heads [8g, 8g+8) through the
attention; the pair (2b, 2b+1) exchanges normalized per-head context via a
pair AllGather, after which BOTH cores of the pair compute the full
[2048, 1024] output projection + residual + LayerNorm for batch b (the
program must be identical across cores, so per-core row slicing is not
possible; the host keeps core 2b's copy).

On-device layout notes:
 - Q^T / K^T are built directly in [head*dk, S] orientation (heads on the
   partition axis, 64 rows per head) so scores^T = K_tile^T . Q and the
   ctx matmul consume them without any transposes.
 - scores are computed transposed, [Sk partition, Sq free], so softmax's
   denominator comes out of the ctx matmul for free: V is stored with a
   ones-column appended per head ([Sk, 65]), so ctx_psum row 64 is the
   softmax denominator.
 - softmax skips the max-subtraction (scores are O(3), exp is safe in fp32,
   masked entries get -1e9 pre-exp and underflow to exactly 0).
 - causality: scores^T tiles are only computed on the valid triangle; the
   single diagonal 128x128 block per k-tile gets a -1e9 additive mask.
"""

import numpy as np
from contextlib import ExitStack

import concourse.bass as bass
import concourse.mybir as mybir
import concourse.tile as tile
import bass_rust
from concourse.tile import ScopedClock
from concourse.bass_utils import run_bass_kernel_spmd

FP = mybir.dt.float32
AF = mybir.ActivationFunctionType

B, S, D, H, DK = 4, 2048, 1024, 16, 64
N_CORES = 8
HPC = H // 2          # heads per core = 8
NEG = -1e9
EPS = 1e-6

# ---------------------------------------------------------------------------
# Compat shims: this walrus build rejects instructions with more than one
# sync-wait condition; split extra waits onto same-engine NoOp carriers.
# ---------------------------------------------------------------------------
_ws_ctr = [0]


def _split_waits_in_ordered(ordered):
    for bb_name, insts in list(ordered.items()):
        new = []
        for inst in insts:
            si = inst.sync_info
            if si is None:
                new.append(inst)
                continue
            waits = list(si.on_wait)
            if len(waits) > 1:
                head = len(waits) - 1
                for i in range(head):
                    _ws_ctr[0] += 1
                    carrier = mybir.InstNoOp(
                        name=f"I-ws{_ws_ctr[0]}", engine=inst.engine
                    )
                    carrier.sync_info = bass_rust.SyncInfo(
                        on_wait=[waits[i]], on_update=[]
                    )
                    new.append(carrier)
                inst.sync_info = bass_rust.SyncInfo(
                    on_wait=waits[head:], on_update=si.on_update
                )
            new.append(inst)
        ordered[bb_name] = new


_orig_lower = tile.TileContext._lower_ordered_insts


def _patched_lower(self, ordered):
    _split_waits_in_ordered(ordered)
    return _orig_lower(self, ordered)


def _split_drain_and_barrier(self, tick_clock, wait_clock):
    drain_inst = self.nc.sync.drain()
    wait_clock.add_sem_waits(
        drain_inst.ins, ScopedClock({None: tick_clock.global_clock})
    )
    si = drain_inst.ins.sync_info
    waits = list(si.on_wait)
    if len(waits) > 1:
        drain_inst.ins.sync_info = bass_rust.SyncInfo(
            on_wait=waits[:1], on_update=si.on_update
        )
        for i in range(1, len(waits)):
            d2 = self.nc.sync.drain()
            d2.ins.sync_info = bass_rust.SyncInfo(
                on_wait=[waits[i]], on_update=[]
            )
    self.nc.all_engine_barrier()
    assert self.sems is not None
    popped = self.nc._tile_sem_poison_stack.pop()
    assert popped is self._sem_poison
    self.nc.clear_and_free_semaphores(list(self.sems.allocated().values()))
    self.nc.all_engine_barrier()


def _install_compat():
    tile.TileContext._lower_ordered_insts = _patched_lower
    tile.TileContext._drain_and_barrier = _split_drain_and_barrier


# ---------------------------------------------------------------------------
# Program builder
# ---------------------------------------------------------------------------
_cached_nc = None


def _phase12_scope(tc):
    # plain pool handle wrapped so ExitStack never double-releases it
    import contextlib

    @contextlib.contextmanager
    def scope():
        pool = tc.alloc_tile_pool(name="pqkv", bufs=1)
        try:
            yield pool
        finally:
            if not pool._released:
                pool.release()
    return scope()


def _ap(tensor, offset, dims):
    return bass.AP(tensor=tensor, offset=offset, ap=[list(d) for d in dims])


def build_nc():
    global _cached_nc
    if _cached_nc is not None:
        return _cached_nc
    _install_compat()
    nc = bass.Bass("TRN2", target_bir_lowering=False, debug=False,
                   num_devices=N_CORES)

    xT = nc.dram_tensor("xT", [D, S], FP, kind="ExternalInput")
    xres = nc.dram_tensor("xres", [S, D], FP, kind="ExternalInput")
    wq = nc.dram_tensor("wq", [D, 512], FP, kind="ExternalInput")
    wk = nc.dram_tensor("wk", [D, 512], FP, kind="ExternalInput")
    wv = nc.dram_tensor("wv", [D, 512], FP, kind="ExternalInput")
    bq = nc.dram_tensor("bq", [512], FP, kind="ExternalInput")
    bk = nc.dram_tensor("bk", [512], FP, kind="ExternalInput")
    bv = nc.dram_tensor("bv", [512], FP, kind="ExternalInput")
    wo = nc.dram_tensor("wo", [D, D], FP, kind="ExternalInput")
    gamma = nc.dram_tensor("gamma", [D], FP, kind="ExternalInput")
    beta = nc.dram_tensor("beta", [D], FP, kind="ExternalInput")
    mneg = nc.dram_tensor("mneg", [128, 128], FP, kind="ExternalInput")
    yout = nc.dram_tensor("y", [S, D], FP, kind="ExternalOutput")

    NKT = S // 128            # 16 k-tiles over the sequence

    with tile.TileContext(nc) as tc:
        with ExitStack() as ctx:
            dram = ctx.enter_context(
                tc.tile_pool(name="dram", bufs=1, space="DRAM"))

            pqkv = ctx.enter_context(_phase12_scope(tc))
            QT = pqkv.tile([128, 4, S], FP)    # [hd%128, hd//128, s]
            KT = pqkv.tile([128, 4, S], FP)
            V = pqkv.tile([128, NKT, HPC, 65], FP)  # per-head V + ones col
            mneg_t = pqkv.tile([128, 128], FP)
            nc.sync.dma_start(out=mneg_t, in_=mneg[:, :])
            nc.vector.memset(V[:, :, :, 64:65], 1.0)

            # ---------------- phase 1: Q^T, K^T, V projections -------------
            with tc.tile_pool(name="p1w", bufs=1) as p1w, \
                 tc.tile_pool(name="p1x", bufs=2) as p1x, \
                 tc.tile_pool(name="p1ps", bufs=4, space="PSUM") as p1ps:
                wq_t = p1w.tile([128, 8, 512], FP)
                wk_t = p1w.tile([128, 8, 512], FP)
                wv_t = p1w.tile([128, 8, 512], FP)
                for wt, wd in ((wq_t, wq), (wk_t, wk), (wv_t, wv)):
                    nc.sync.dma_start(
                        out=wt, in_=_ap(wd, 0, [[512, 128], [512 * 128, 8],
                                                [1, 512]]))
                bq_t = p1w.tile([128, 4], FP)
                bk_t = p1w.tile([128, 4], FP)
                nc.sync.dma_start(out=bq_t, in_=_ap(bq, 0, [[1, 128], [128, 4]]))
                nc.sync.dma_start(out=bk_t, in_=_ap(bk, 0, [[1, 128], [128, 4]]))
                bv_bc = p1w.tile([128, 8, 64], FP)
                nc.sync.dma_start(
                    out=bv_bc, in_=_ap(bv, 0, [[0, 128], [64, 8], [1, 64]]))

                for sq in range(4):               # S in quarters of 512
                    xq = p1x.tile([128, 8, 512], FP, name="xq")
                    nc.sync.dma_start(
                        out=xq, in_=_ap(xT, 512 * sq,
                                        [[S, 128], [128 * S, 8], [1, 512]]))
                    for wt, bt, dst in ((wq_t, bq_t, QT), (wk_t, bk_t, KT)):
                        for mt in range(4):
                            ps = p1ps.tile([128, 512], FP, name="ps1",
                                           tag="ps1")
                            for d in range(8):
                                nc.tensor.matmul(
                                    ps, wt[:, d, 128 * mt:128 * (mt + 1)],
                                    xq[:, d, :],
                                    start=(d == 0), stop=(d == 7))
                            nc.vector.tensor_scalar_add(
                                dst[:, mt, 512 * sq:512 * (sq + 1)], ps,
                                bt[:, mt:mt + 1])
                    for st in range(4):
                        ps = p1ps.tile([128, 512], FP, name="ps1v", tag="ps1")
                        for d in range(8):
                            nc.tensor.matmul(
                                ps, xq[:, d, 128 * st:128 * (st + 1)],
                                wv_t[:, d, :],
                                start=(d == 0), stop=(d == 7))
                        sg = 4 * sq + st
                        nc.vector.tensor_add(
                            V[:, sg, :, 0:64],
                            ps.rearrange("p (h e) -> p h e", h=HPC), bv_bc)

            # ---------------- phase 2: attention per head-pair -------------
            pctx = tc.alloc_tile_pool(name="pctx", bufs=1)
            ctxT = pctx.tile([128, 4, S], FP)     # normalized ctx^T, packed

            agouts = []
            with tc.tile_pool(name="p2e", bufs=2) as p2e, \
                 tc.tile_pool(name="p2u", bufs=2) as p2u, \
                 tc.tile_pool(name="p2m", bufs=1) as p2m, \
                 tc.tile_pool(name="p2sS", bufs=1, space="PSUM") as p2sS, \
                 tc.tile_pool(name="p2sC", bufs=4, space="PSUM") as p2sC:
                for p in range(4):
                    ctxU = [p2u.tile([65, S], FP, name=f"ctxU{h}",
                                     tag="ctxU") for h in range(2)]
                    for w in range(2):
                        qlo = 1024 * w
                        kmax = 8 * (w + 1)
                        cps = {}
                        for h in range(2):
                            for jl in range(2):
                                cps[h, jl] = p2sC.tile(
                                    [65, 512], FP, name=f"cps{h}{jl}",
                                    tag="cps")
                        for k in range(kmax):
                            clo = max(qlo, 128 * k)
                            span = qlo + 1024 - clo
                            s2 = p2sS.tile([128, 2048], FP, name="s2",
                                           tag="s2")
                            s2v = s2.rearrange("p (h q) -> p h q", h=2)
                            for h in range(2):
                                rows = slice(64 * h, 64 * h + 64)
                                off = 0
                                while off < span:
                                    n = min(512, span - off)
                                    nc.tensor.matmul(
                                        s2[:, 1024 * h + off:1024 * h + off + n],
                                        KT[rows, p, 128 * k:128 * (k + 1)],
                                        QT[rows, p, clo + off:clo + off + n],
                                        start=True, stop=True,
                                        tile_position=(64 * h, 0))
                                    off += n
                            if clo == 128 * k:
                                # diagonal block at column offset 0
                                dv = s2v[:, :, 0:128]
                                mb = _ap(mneg_t.tensor, mneg_t.offset,
                                         [mneg_t.ap[0], [0, 2], mneg_t.ap[1]])
                                nc.vector.tensor_add(dv, dv, mb)
                            expS = p2e.tile([128, 2, 1024], FP, name="expS",
                                            tag="expS")
                            nc.scalar.activation(
                                expS[:, :, 0:span], s2v[:, :, 0:span], AF.Exp)
                            for h in range(2):
                                HH = 2 * p + h
                                for jl in range(2):
                                    jlo = qlo + 512 * jl
                                    if jlo + 512 <= clo:
                                        continue
                                    boff = max(clo, jlo) - clo
                                    doff = max(clo, jlo) - jlo
                                    blen = jlo + 512 - max(clo, jlo)
                                    klast = (jlo + 512) // 128 - 1
                                    nc.tensor.matmul(
                                        cps[h, jl][:, doff:doff + blen],
                                        V[:, k, HH, :],
                                        expS[:, h, boff:boff + blen],
                                        start=(k == 0),
                                        stop=(k == min(klast, kmax - 1)))
                        for h in range(2):
                            for jl in range(2):
                                nc.vector.tensor_copy(
                                    ctxU[h][:, qlo + 512 * jl:
                                            qlo + 512 * (jl + 1)],
                                    cps[h, jl])
                    # ---- normalize pair p and pack into ctxT[:, p, :]
                    den = p2m.tile([2, S], FP, name="den", tag="den")
                    nc.sync.dma_start(out=den[0:1, :], in_=ctxU[0][64:65, :])
                    nc.sync.dma_start(out=den[1:2, :], in_=ctxU[1][64:65, :])
                    rec = p2m.tile([2, S], FP, name="rec", tag="rec")
                    nc.vector.reciprocal(rec, den)
                    recd = dram.tile([2, S], FP, name=f"recd{p}")
                    nc.sync.dma_start(out=recd[:, :], in_=rec)
                    bca = p2m.tile([64, 2, S], FP, name="bca", tag="bca")
                    for h in range(2):
                        nc.sync.dma_start(
                            out=bca[:, h, :],
                            in_=_ap(recd.tensor, recd.offset + h * S, [[0, 64], [1, S]]))
                    for h in range(2):
                        nc.vector.tensor_mul(
                            ctxT[64 * h:64 * (h + 1), p, :],
                            ctxU[h][0:64, :], bca[:, h, :])
                    # ---- pair AllGather of this pair's packed ctx^T chunk
                    agin = dram.tile([128, S], FP, name=f"agin{p}")
                    nc.sync.dma_start(out=agin[:, :], in_=ctxT[:, p, :])
                    agout = dram.tile([256, S], FP, name=f"agout{p}")
                    nc.gpsimd.collective_compute(
                        "AllGather", mybir.AluOpType.bypass,
                        replica_groups=[[0, 1], [2, 3], [4, 5], [6, 7]],
                        ins=[agin.opt()], outs=[agout.opt()])
                    agouts.append(agout)

            pctx.release()
            pqkv.release()

            # ---------------- phase 3: out-proj + residual + LayerNorm -----
            with tc.tile_pool(name="p3c", bufs=1) as p3c, \
                 tc.tile_pool(name="p3x", bufs=3) as p3x, \
                 tc.tile_pool(name="p3ps", bufs=4, space="PSUM") as p3ps:
                wo_t = p3c.tile([128, 8, D], FP)
                nc.sync.dma_start(
                    out=wo_t, in_=_ap(wo, 0, [[D, 128], [128 * D, 8], [1, D]]))
                gam = p3c.tile([128, D], FP)
                bet = p3c.tile([128, D], FP)
                nc.sync.dma_start(out=gam, in_=_ap(gamma, 0, [[0, 128], [1, D]]))
                nc.sync.dma_start(out=bet, in_=_ap(beta, 0, [[0, 128], [1, D]]))
                eps_t = p3c.tile([128, 1], FP)
                nc.vector.memset(eps_t, EPS)
                ctxAG = p3c.tile([128, 8, S], FP)  # kt slot 4g+p: heads 2kt,2kt+1
                for g in range(2):
                    for p in range(4):
                        nc.sync.dma_start(
                            out=ctxAG[:, 4 * g + p, :],
                            in_=agouts[p][128 * g:128 * (g + 1), :])

                for st in range(S // 128):
                    xr = p3x.tile([128, D], FP, name="xr", tag="xr")
                    nc.sync.dma_start(
                        out=xr, in_=xres[128 * st:128 * (st + 1), :])
                    yt = p3x.tile([128, D], FP, name="yt", tag="yt")
                    for dsl in range(2):
                        ps = p3ps.tile([128, 512], FP, name="ps3", tag="ps3")
                        for kt in range(8):
                            nc.tensor.matmul(
                                ps, ctxAG[:, kt, 128 * st:128 * (st + 1)],
                                wo_t[:, kt, 512 * dsl:512 * (dsl + 1)],
                                start=(kt == 0), stop=(kt == 7))
                        nc.vector.tensor_add(
                            yt[:, 512 * dsl:512 * (dsl + 1)], ps,
                            xr[:, 512 * dsl:512 * (dsl + 1)])
                    stats = p3x.tile([128, 2, 6], FP, name="stats",
                                     tag="stats")
                    for hhalf in range(2):
                        nc.vector.bn_stats(
                            stats[:, hhalf, :],
                            yt[:, 512 * hhalf:512 * (hhalf + 1)])
                    mv = p3x.tile([128, 2], FP, name="mv", tag="mv")
                    nc.vector.bn_aggr(mv, stats)
                    ut = p3x.tile([128, D], FP, name="ut", tag="ut")
                    nc.vector.tensor_scalar_sub(ut, yt, mv[:, 0:1])
                    sd = p3x.tile([128, 1], FP, name="sd", tag="sd")
                    nc.scalar.activation(sd, mv[:, 1:2], AF.Sqrt,
                                         bias=eps_t, scale=1.0)
                    rstd = p3x.tile([128, 1], FP, name="rstd", tag="rstd")
                    nc.vector.reciprocal(rstd, sd)
                    nc.vector.tensor_scalar_mul(ut, ut, rstd)
                    nc.vector.tensor_mul(ut, ut, gam)
                    ot = p3x.tile([128, D], FP, name="ot", tag="ot")
                    nc.vector.tensor_add(ot, ut, bet)
                    nc.sync.dma_start(
                        out=yout[128 * st:128 * (st + 1), :], in_=ot)

    _cached_nc = nc
    return nc


# ---------------------------------------------------------------------------
# Host-side entry point
# ---------------------------------------------------------------------------
def make_in_maps(x, Wq, bq, Wk, bk, Wv, bv, Wo, bo, gamma, beta):
    x = np.asarray(x, np.float32)
    WqS = (np.asarray(Wq, np.float32) / np.sqrt(np.float32(DK))).reshape(D, H * DK)
    bqS = (np.asarray(bq, np.float32) / np.sqrt(np.float32(DK))).reshape(H * DK)
    WkF = np.asarray(Wk, np.float32).reshape(D, H * DK)
    bkF = np.asarray(bk, np.float32).reshape(H * DK)
    WvF = np.asarray(Wv, np.float32).reshape(D, H * DK)
    bvF = np.asarray(bv, np.float32).reshape(H * DK)
    WoF = np.ascontiguousarray(np.asarray(Wo, np.float32).reshape(H * DK, D))
    boF = np.asarray(bo, np.float32)
    gF = np.ascontiguousarray(np.asarray(gamma, np.float32))
    btF = np.ascontiguousarray(np.asarray(beta, np.float32))
    kk = np.arange(128)[:, None]
    qq = np.arange(128)[None, :]
    mneg = np.where(kk <= qq, 0.0, NEG).astype(np.float32)

    in_maps = []
    for c in range(N_CORES):
        b, g = divmod(c, 2)
        cols = slice(512 * g, 512 * (g + 1))
        in_maps.append({
            "xT": np.ascontiguousarray(x[b].T),
            "xres": np.ascontiguousarray(x[b] + boF[None, :]),
            "wq": np.ascontiguousarray(WqS[:, cols]),
            "wk": np.ascontiguousarray(WkF[:, cols]),
            "wv": np.ascontiguousarray(WvF[:, cols]),
            "bq": np.ascontiguousarray(bqS[cols]),
            "bk": np.ascontiguousarray(bkF[cols]),
            "bv": np.ascontiguousarray(bvF[cols]),
            "wo": WoF,
            "gamma": gF,
            "beta": btF,
            "mneg": mneg,
        })
    return in_maps


def kernel(x, Wq, bq, Wk, bk, Wv, bv, Wo, bo, gamma, beta):
    nc = build_nc()
    in_maps = make_in_maps(x, Wq, bq, Wk, bk, Wv, bv, Wo, bo, gamma, beta)
    r = run_bass_kernel_spmd(nc, in_maps, list(range(N_CORES)))
    out = np.empty((B, S, D), np.float32)
    for b in range(B):
        out[b] = r.results[2 * b]["y"]
    return out
